# revision 20
# baseline (speedup 1.0000x reference)
"""Multi-head latent attention (MLA) forward pass on 8 Trainium2 NeuronCores.

Sharding: 2 (batch) x 4 (head-group) grid. Core c handles batch b = c // 4
and heads 4*(c % 4) .. 4*(c % 4) + 3.  Each core:
  - streams x[b]^T (host-pretransposed, bf16) once
  - GEMM-A: A = x_b @ [Wq_heads | Wkrope_heads | Wkv_down]   (combined)
  - batched rms-norms (12 64-wide groups + c_kv in one op-chain per s-tile);
    1/sqrt(m) and 1/l computed as exp(-a*ln(.)) on ScalarE (one table set)
  - rope on q/k_rope; PE-transposes into head-dim-major layout
  - GEMM-3 (kv_up) interleaved per s-tile
  - causal attention per head in "transposed-score" form:
      S^T[k,q] = K^T.T @ Q^T ; P^T = exp(S^T/sqrt(HD)) * mask ;
      l[q] = ones.T @ P^T ;  y^T[d,q] = V.T @ P^T ; y^T *= 1/l (bcast)
  - proj partial: out_b += y^T.T @ Wp_rows  (interleaved with attention)
Host sums the 4 partials per batch element.
"""

import sys

for _p in ("/opt/trn_rl_repo",):
    if _p not in sys.path:
        sys.path.insert(0, _p)

import math
from contextlib import ExitStack

import ml_dtypes
import numpy as np

import concourse.bass as bass
import concourse.mybir as mybir
import concourse.tile as tile
from concourse import bacc
from concourse.bass_utils import run_bass_kernel_spmd

F32 = mybir.dt.float32
BF16 = mybir.dt.bfloat16
BF = ml_dtypes.bfloat16

B, S, D = 2, 2048, 2048
H = 16
HD = 128           # head dim
ROPE = 64
NOPE = 64
LAT = 512
EPS = 1e-6
ROPE_BASE = 10000.0

H_LOC = 4          # heads per core
N_CORES = 8
DLOC = H_LOC * HD  # 512, per-core proj contraction size

ST_N = S // 128    # 16 s-tiles
KT_N = D // 128    # 16 k-tiles for GEMM-A
QB = 512           # attention q-block width
NB = 512           # proj output block width

A_QW = H_LOC * HD             # 512  q columns in A
A_RW = H_LOC * ROPE           # 256  k_rope columns in A
A_W = A_QW + A_RW + LAT       # 1280 total A columns
KV_W = H_LOC * NOPE + H_LOC * HD   # 768 kv columns

X8_CHUNK = 256                # s-columns of x^T per streamed chunk
X8_N = S // X8_CHUNK          # 8 chunks
ST_PER_CHUNK = X8_CHUNK // 128  # 2

MULT = mybir.AluOpType.mult
ADD = mybir.AluOpType.add
SUB = mybir.AluOpType.subtract
EXPF = mybir.ActivationFunctionType.Exp
LNF = mybir.ActivationFunctionType.Ln
SQRTF = mybir.ActivationFunctionType.Sqrt
SQF = mybir.ActivationFunctionType.Square
AXX = mybir.AxisListType.X
AXXY = mybir.AxisListType.XY

_PROGRAM_CACHE = {}


def _build_program():
    nc = bacc.Bacc(None, target_bir_lowering=False, debug=True)

    # ---- DRAM I/O ----
    xT8 = nc.dram_tensor("xT8", [X8_N, D, X8_CHUNK], BF16, kind="ExternalInput")
    w_a = nc.dram_tensor("w_a", [D, A_W], BF16, kind="ExternalInput")
    w_up = nc.dram_tensor("w_up", [LAT, KV_W], BF16, kind="ExternalInput")
    w_p = nc.dram_tensor("w_p", [DLOC, D], BF16, kind="ExternalInput")
    cos4 = nc.dram_tensor("cos4", [S, H_LOC, ROPE // 2], BF16, kind="ExternalInput")
    sin4 = nc.dram_tensor("sin4", [S, H_LOC, ROPE // 2], BF16, kind="ExternalInput")
    masks = nc.dram_tensor("masks", [128, 4, QB], BF16, kind="ExternalInput")
    gain13 = nc.dram_tensor("gain13", [128, 13], F32, kind="ExternalInput")
    ones_in = nc.dram_tensor("ones_in", [128, 1], BF16, kind="ExternalInput")
    ident_in = nc.dram_tensor("ident_in", [128, 128], BF16, kind="ExternalInput")
    out = nc.dram_tensor("out", [S, D], F32, kind="ExternalOutput")

    with tile.TileContext(nc) as tc, ExitStack() as top:
        const = top.enter_context(tc.tile_pool(name="const", bufs=1))
        big = top.enter_context(tc.tile_pool(name="big", bufs=1))

        # --- resident weights/constants, load order = consumption order ---
        wa_sb = const.tile([128, KT_N, A_W], BF16)
        wa_r = w_a[:].rearrange("(k p) n -> p k n", p=128)
        nc.sync.dma_start(out=wa_sb[:, 0, :], in_=wa_r[:, 0, :])
        wup_sb = const.tile([128, LAT // 128, KV_W], BF16)
        nc.sync.dma_start(out=wup_sb[:], in_=w_up[:].rearrange("(k p) n -> p k n", p=128))
        cos_sb = const.tile([128, ST_N, H_LOC, ROPE // 2], BF16)
        nc.sync.dma_start(out=cos_sb[:], in_=cos4[:].rearrange("(t p) h f -> p t h f", p=128))
        sin_sb = const.tile([128, ST_N, H_LOC, ROPE // 2], BF16)
        nc.sync.dma_start(out=sin_sb[:], in_=sin4[:].rearrange("(t p) h f -> p t h f", p=128))
        gain_sb = const.tile([128, 13], F32)
        nc.sync.dma_start(out=gain_sb[:], in_=gain13[:])
        ones_sb = const.tile([128, 1], BF16)
        nc.sync.dma_start(out=ones_sb[:], in_=ones_in[:])
        ident_sb = const.tile([128, 128], BF16)
        nc.sync.dma_start(out=ident_sb[:], in_=ident_in[:])
        eps_sb = const.tile([128, 1], F32)
        nc.vector.memset(eps_sb[:], EPS)
        # loaded late (only needed by attention/proj phase)
        mask_sb = const.tile([128, 4, QB], BF16)
        wp_sb = const.tile([128, H_LOC, D], BF16)

        # --- persistent activations (head-dim-major) ---
        QT = big.tile([128, H_LOC, S], BF16)   # [d, h, q]
        KT = big.tile([128, H_LOC, S], BF16)   # [d, h, k] (0:64 nope, 64:128 rope)
        V = big.tile([128, ST_N, H_LOC * HD], BF16)  # [s%128, s//128, d_loc]

        # ===== phase 1 scratch =====
        p12 = ExitStack()
        ckvT_pool = p12.enter_context(tc.tile_pool(name="ckvT_pool", bufs=1))
        x8p = p12.enter_context(tc.tile_pool(name="x8p", bufs=2))
        xq0 = x8p.tile([128, KT_N, X8_CHUNK], BF16, tag="x8")
        nc.sync.dma_start(out=xq0[:], in_=xT8[0].rearrange("(k p) s -> p k s", p=128))
        for kt in range(1, KT_N):
            nc.sync.dma_start(out=wa_sb[:, kt, :], in_=wa_r[:, kt, :])
        scr = p12.enter_context(tc.tile_pool(name="scr", bufs=2))
        jnk = p12.enter_context(tc.tile_pool(name="jnk", bufs=2))
        ckvT = ckvT_pool.tile([128, LAT // 128, S], BF16)  # [lat, lt, s]

        def rope_rot(xn, cos_ap, sin_ap, out1, out2):
            """out1 = x1*c + x2*s ; out2 = x2*c - x1*s  (4-head wide)."""
            RH = ROPE // 2
            x1 = xn[:, :, 0:RH]
            x2 = xn[:, :, RH:ROPE]
            t1 = scr.tile([128, H_LOC, RH], F32, tag="t1")
            t2 = scr.tile([128, H_LOC, RH], F32, tag="t2")
            nc.vector.tensor_tensor(t1[:], x1, cos_ap, MULT)
            nc.vector.tensor_tensor(t2[:], x2, sin_ap, MULT)
            nc.vector.tensor_tensor(out1, t1[:], t2[:], ADD)
            t3 = scr.tile([128, H_LOC, RH], F32, tag="t3")
            t4 = scr.tile([128, H_LOC, RH], F32, tag="t4")
            nc.vector.tensor_tensor(t3[:], x2, cos_ap, MULT)
            nc.vector.tensor_tensor(t4[:], x1, sin_ap, MULT)
            nc.vector.tensor_tensor(out2, t3[:], t4[:], SUB)

        def rsqrt_act(dst, src, n):
            """dst = 1/sqrt(src/n + eps): ACT Sqrt then fast DVE reciprocal."""
            nc.scalar.activation(dst, src, SQRTF, scale=1.0 / n, bias=eps_sb[:])
            nc.vector.reciprocal_approx_fast(out=dst, in_=dst)

        # ========== phase 1: GEMM-A + norms + rope + GEMM-3, per s-tile =====
        with (
            tc.tile_pool(name="psA", bufs=2, space="PSUM") as psA,
            tc.tile_pool(name="psT", bufs=2, space="PSUM") as psT,
        ):
            for e in range(X8_N):
                if e == 0:
                    xq = xq0
                else:
                    xq = x8p.tile([128, KT_N, X8_CHUNK], BF16, tag="x8")
                    nc.sync.dma_start(
                        out=xq[:], in_=xT8[e].rearrange("(k p) s -> p k s", p=128))
                for st2 in range(ST_PER_CHUNK):
                    ST = e * ST_PER_CHUNK + st2
                    s0 = ST * 128
                    aps = psA.tile([128, A_W], F32, tag="A")
                    for kt in range(KT_N):
                        lhs = xq[:, kt, st2 * 128:(st2 + 1) * 128]
                        for c0, c1 in ((0, 512), (512, 1024), (1024, 1280)):
                            nc.tensor.matmul(
                                aps[:, c0:c1], lhs, wa_sb[:, kt, c0:c1],
                                start=(kt == 0), stop=(kt == KT_N - 1))

                    # ---- evict + batched stats ----
                    asb = scr.tile([128, A_W], F32, tag="asb")
                    nc.scalar.copy(asb[:], aps[:])
                    junk = jnk.tile([128, A_W], BF16, tag="junk")
                    nc.scalar.activation(junk[:], aps[:], SQF)
                    rs13 = scr.tile([128, 13], F32, tag="rs13")
                    nc.vector.tensor_reduce(
                        rs13[:, 0:12],
                        junk[:, 0:768].rearrange("p (g c) -> p g c", c=64),
                        AXX, ADD)
                    nc.vector.tensor_reduce(
                        rs13[:, 12:13],
                        junk[:, 768:1280].rearrange("p (g c) -> p g c", c=64),
                        AXXY, ADD)
                    rsqrt_act(rs13[:, 0:12], rs13[:, 0:12], 64)
                    rsqrt_act(rs13[:, 12:13], rs13[:, 12:13], LAT)
                    nc.vector.tensor_tensor(rs13[:], rs13[:], gain_sb[:], MULT)

                    # ---- apply norms ----
                    nrm = scr.tile([128, 768], BF16, tag="nrm")
                    nc.vector.tensor_tensor(
                        nrm[:].rearrange("p (g c) -> p g c", c=64),
                        asb[:, 0:768].rearrange("p (g c) -> p g c", c=64),
                        rs13[:, 0:12].to_broadcast([128, 12, 64]), MULT)
                    cv = scr.tile([128, LAT], BF16, tag="cv")
                    nc.vector.tensor_scalar(
                        cv[:], asb[:, 768:1280], rs13[:, 12:13], None, MULT)

                    # ---- rope ----
                    nrmq = nrm[:, 0:512].rearrange("p (h t c) -> p h t c", t=2, c=64)
                    qno = nrmq[:, :, 0, :]
                    qro = nrmq[:, :, 1, :]
                    kro = nrm[:, 512:768].rearrange("p (h c) -> p h c", c=64)
                    qrot = scr.tile([128, H_LOC, ROPE], BF16, tag="qrot")
                    krot = scr.tile([128, H_LOC, ROPE], BF16, tag="krot")
                    RH = ROPE // 2
                    rope_rot(qro, cos_sb[:, ST], sin_sb[:, ST],
                             qrot[:, :, 0:RH], qrot[:, :, RH:ROPE])
                    rope_rot(kro, cos_sb[:, ST], sin_sb[:, ST],
                             krot[:, :, 0:RH], krot[:, :, RH:ROPE])

                    # ---- transposes (q/krope/ckv) ----
                    for h in range(H_LOC):
                        tq = psT.tile([128, 128], BF16, tag="tq")
                        nc.tensor.transpose(tq[0:64, :], qno[:, h, :], ident_sb[:])
                        nc.tensor.transpose(tq[64:128, :], qrot[:, h, :], ident_sb[:])
                        nc.scalar.copy(QT[:, h, s0:s0 + 128], tq[:])
                        tk = psT.tile([128, 128], BF16, tag="tq")
                        nc.tensor.transpose(tk[64:128, :], krot[:, h, :], ident_sb[:])
                        nc.scalar.copy(KT[64:128, h, s0:s0 + 128], tk[64:128, :])
                    for lt in range(LAT // 128):
                        tcv = psT.tile([128, 128], BF16, tag="tq")
                        nc.tensor.transpose(
                            tcv[:], cv[:, lt * 128:(lt + 1) * 128], ident_sb[:])
                        nc.scalar.copy(ckvT[:, lt, s0:s0 + 128], tcv[:])

        # ================= phase 2: GEMM-3 (kv_up) =================
        with (
            tc.tile_pool(name="psKV", bufs=2, space="PSUM") as psKV,
            tc.tile_pool(name="psT2", bufs=2, space="PSUM") as psT2,
        ):
            for ST in range(ST_N):
                s0 = ST * 128
                kvps = psKV.tile([128, KV_W], F32, tag="KV")
                for lt in range(LAT // 128):
                    lhs = ckvT[:, lt, s0:s0 + 128]
                    for c0, c1 in ((0, 512), (512, 768)):
                        nc.tensor.matmul(
                            kvps[:, c0:c1], lhs, wup_sb[:, lt, c0:c1],
                            start=(lt == 0), stop=(lt == LAT // 128 - 1))
                # k_nope batched norm + transpose into KT[0:64]
                kvev = scr.tile([128, 256], F32, tag="kvev")
                nc.scalar.copy(kvev[:], kvps[:, 0:256])
                junkk = jnk.tile([128, 256], BF16, tag="junkk")
                nc.scalar.activation(junkk[:], kvps[:, 0:256], SQF)
                rsk = scr.tile([128, 4], F32, tag="rsk")
                nc.vector.tensor_reduce(
                    rsk[:], junkk[:].rearrange("p (g c) -> p g c", c=64),
                    AXX, ADD)
                rsqrt_act(rsk[:], rsk[:], 64)
                knrm = scr.tile([128, H_LOC, NOPE], BF16, tag="knrm")
                nc.vector.tensor_tensor(
                    knrm[:],
                    kvev[:].rearrange("p (g c) -> p g c", c=64),
                    rsk[:].to_broadcast([128, 4, 64]), MULT)
                for h in range(H_LOC):
                    tkn = psT2.tile([128, 128], BF16, tag="tkn")
                    nc.tensor.transpose(tkn[0:64, :], knrm[:, h, :], ident_sb[:])
                    nc.vector.tensor_copy(
                        KT[0:64, h, s0:s0 + 128], tkn[0:64, :])
                # V evict (ACT copy, table-free)
                nc.scalar.copy(V[:, ST, :], kvps[:, H_LOC * NOPE:KV_W])

        p12.close()

        # late const loads (attention/proj only)
        nc.sync.dma_start(out=mask_sb[:], in_=masks[:])
        nc.sync.dma_start(out=wp_sb[:], in_=w_p[:].rearrange("(k p) n -> p k n", p=128))

        # ============ phase 3: attention + out projection (interleaved) ======
        yT_pool = top.enter_context(tc.tile_pool(name="yT_pool", bufs=1))
        yT = yT_pool.tile([128, H_LOC, S], BF16)  # [d, h, q]
        inv_sqrt_hd = 1.0 / math.sqrt(HD)
        with (
            tc.tile_pool(name="pP", bufs=6) as pP,
            tc.tile_pool(name="pR", bufs=2) as pR,
            tc.tile_pool(name="psS", bufs=4, space="PSUM") as psS,
            tc.tile_pool(name="psL", bufs=2, space="PSUM") as psL,
            tc.tile_pool(name="psY", bufs=2, space="PSUM") as psY,
        ):
            for j in range(S // QB):
                q0 = j * QB
                nkt = (q0 + QB) // 128
                for hp in range(H_LOC // 2):
                    hh = (2 * hp, 2 * hp + 1)
                    lps = {}
                    yps = {}
                    for h in hh:
                        lp_t = psL.tile([1, QB], F32, tag="L")
                        yp_t = psY.tile([128, QB], F32, tag="Y")
                        lps[h] = lp_t
                        yps[h] = yp_t
                    for kt in range(nkt):
                        Ps = {}
                        for h in hh:
                            sps = psS.tile([128, QB], F32, tag="Ssc")
                            nc.tensor.matmul(
                                sps[:], KT[:, h, kt * 128:(kt + 1) * 128],
                                QT[:, h, q0:q0 + QB], start=True, stop=True)
                            P = pP.tile([128, QB], BF16, tag="P")
                            nc.scalar.activation(
                                P[:], sps[:], EXPF, scale=inv_sqrt_hd)
                            d_idx = kt - (q0 // 128)
                            if d_idx >= 0:
                                nc.vector.tensor_tensor(
                                    P[:], P[:], mask_sb[:, d_idx, :], MULT)
                            Ps[h] = P
                        for h in hh:
                            nc.tensor.matmul(
                                lps[h][:], ones_sb[:], Ps[h][:],
                                start=(kt == 0), stop=(kt == nkt - 1))
                            nc.tensor.matmul(
                                yps[h][:], V[:, kt, h * HD:(h + 1) * HD], Ps[h][:],
                                start=(kt == 0), stop=(kt == nkt - 1))
                    for h in hh:
                        r = pR.tile([1, QB], F32, tag="r")
                        nc.vector.reciprocal_approx_fast(out=r[:], in_=lps[h][:])
                        rbc = pR.tile([128, QB], F32, tag="rbc")
                        nc.gpsimd.partition_broadcast(rbc[:], r[:])
                        nc.vector.tensor_tensor(
                            yT[:, h, q0:q0 + QB], yps[h][:], rbc[:], MULT)

        # ============ phase 4: out projection (LDW-amortized order) ==========
        with (
            tc.tile_pool(name="pO", bufs=4) as pO,
            tc.tile_pool(name="psO", bufs=8, space="PSUM") as psO,
        ):
            for ST in range(ST_N):
                s0 = ST * 128
                otiles = []
                for _nb in range(D // NB):
                    ot = psO.tile([128, NB], F32, tag="O")
                    otiles.append(ot)
                for h in range(H_LOC):
                    for nb in range(D // NB):
                        nc.tensor.matmul(
                            otiles[nb][:], yT[:, h, s0:s0 + 128],
                            wp_sb[:, h, nb * NB:(nb + 1) * NB],
                            start=(h == 0), stop=(h == H_LOC - 1))
                for nb in range(D // NB):
                    osb = pO.tile([128, NB], F32, tag="osb")
                    nc.vector.tensor_copy(osb[:], otiles[nb][:])
                    nc.sync.dma_start(
                        out=out[s0:s0 + 128, nb * NB:(nb + 1) * NB], in_=osb[:])
    nc.compile()
    return nc


def _prep_inputs(x, w_q_krope, w_kv_down, w_kv_up, w_proj, q_gain):
    """Build the 8 per-core input maps (host-side sharding)."""
    inv_freq = ROPE_BASE ** (-np.arange(0, ROPE, 2, dtype=np.float32) / ROPE)
    t = np.arange(S, dtype=np.float32)
    freqs = np.outer(t, inv_freq)                      # (S, 32)
    cos4 = np.ascontiguousarray(np.broadcast_to(
        np.cos(freqs)[:, None, :], (S, H_LOC, ROPE // 2))).astype(BF)
    sin4 = np.ascontiguousarray(np.broadcast_to(
        np.sin(freqs)[:, None, :], (S, H_LOC, ROPE // 2))).astype(BF)

    kk = np.arange(128)[:, None, None]
    dd = np.arange(4)[None, :, None]
    qq = np.arange(QB)[None, None, :]
    masks = (kk + 128 * dd <= qq).astype(BF)           # [128, 4, QB]

    ones_in = np.ones((128, 1), dtype=BF)
    ident_in = np.eye(128, dtype=np.float32).astype(BF)

    # x^T per batch, chunked: [X8_N, D, X8_CHUNK]
    xT_chunks = []
    for b in range(B):
        xT = np.ascontiguousarray(x[b].T).astype(BF)   # [D, S]
        xT_chunks.append(np.ascontiguousarray(
            xT.reshape(D, X8_N, X8_CHUNK).transpose(1, 0, 2)))

    in_maps = []
    for c in range(N_CORES):
        b = c // H_LOC
        hg = c % H_LOC
        heads = [hg * H_LOC + i for i in range(H_LOC)]
        w_a = np.concatenate(
            [w_q_krope[:, h * HD:(h + 1) * HD] for h in heads]
            + [w_q_krope[:, D + h * ROPE:D + (h + 1) * ROPE] for h in heads]
            + [w_kv_down], axis=1).astype(BF)           # [D, 1280]
        w_up = np.concatenate(
            [w_kv_up[:, h * NOPE:(h + 1) * NOPE] for h in heads]
            + [w_kv_up[:, NOPE * H + h * HD:NOPE * H + (h + 1) * HD]
               for h in heads], axis=1).astype(BF)      # [LAT, 768]
        w_p = w_proj[hg * DLOC:(hg + 1) * DLOC, :].astype(BF)   # [512, D]
        g = q_gain[heads].astype(np.float32)
        g13 = np.concatenate([np.repeat(g, 2), np.ones(5, np.float32)])
        gain13 = np.ascontiguousarray(
            np.broadcast_to(g13[None, :], (128, 13))).astype(np.float32)
        in_maps.append({
            "xT8": xT_chunks[b],
            "w_a": np.ascontiguousarray(w_a),
            "w_up": np.ascontiguousarray(w_up),
            "w_p": np.ascontiguousarray(w_p),
            "cos4": cos4, "sin4": sin4, "masks": masks,
            "gain13": gain13,
            "ones_in": ones_in, "ident_in": ident_in,
        })
    return in_maps


def kernel(x, w_q_krope, w_kv_down, w_kv_up, w_proj, q_gain, **_unused):
    x = np.asarray(x, dtype=np.float32)
    w_q_krope = np.asarray(w_q_krope, dtype=np.float32)
    w_kv_down = np.asarray(w_kv_down, dtype=np.float32)
    w_kv_up = np.asarray(w_kv_up, dtype=np.float32)
    w_proj = np.asarray(w_proj, dtype=np.float32)
    q_gain = np.asarray(q_gain, dtype=np.float32)

    if "nc" not in _PROGRAM_CACHE:
        _PROGRAM_CACHE["nc"] = _build_program()
    nc = _PROGRAM_CACHE["nc"]

    in_maps = _prep_inputs(x, w_q_krope, w_kv_down, w_kv_up, w_proj, q_gain)
    res = run_bass_kernel_spmd(nc, in_maps, list(range(N_CORES)))

    out = np.zeros((B, S, D), dtype=np.float32)
    for c in range(N_CORES):
        out[c // H_LOC] += res.results[c]["out"]
    return out


# revision 21
# speedup vs baseline: 1.0402x; 1.0402x over previous
"""Multi-head latent attention (MLA) forward pass on 8 Trainium2 NeuronCores.

Sharding: 2 (batch) x 4 (head-group) grid. Core c handles batch b = c // 4
and heads 4*(c % 4) .. 4*(c % 4) + 3.  Each core:
  - streams x[b]^T (host-pretransposed, bf16) once
  - GEMM-A: A = x_b @ [Wq_heads | Wkrope_heads | Wkv_down]   (combined)
  - batched rms-norms (12 64-wide groups + c_kv in one op-chain per s-tile);
    1/sqrt(m) and 1/l computed as exp(-a*ln(.)) on ScalarE (one table set)
  - rope on q/k_rope; PE-transposes into head-dim-major layout
  - GEMM-3 (kv_up) interleaved per s-tile
  - causal attention per head in "transposed-score" form:
      S^T[k,q] = K^T.T @ Q^T ; P^T = exp(S^T/sqrt(HD)) * mask ;
      l[q] = ones.T @ P^T ;  y^T[d,q] = V.T @ P^T ; y^T *= 1/l (bcast)
  - proj partial: out_b += y^T.T @ Wp_rows  (interleaved with attention)
Host sums the 4 partials per batch element.
"""

import sys

for _p in ("/opt/trn_rl_repo",):
    if _p not in sys.path:
        sys.path.insert(0, _p)

import math
from contextlib import ExitStack

import ml_dtypes
import numpy as np

import concourse.bass as bass
import concourse.mybir as mybir
import concourse.tile as tile
from concourse import bacc
from concourse.bass_utils import run_bass_kernel_spmd

F32 = mybir.dt.float32
BF16 = mybir.dt.bfloat16
BF = ml_dtypes.bfloat16

B, S, D = 2, 2048, 2048
H = 16
HD = 128           # head dim
ROPE = 64
NOPE = 64
LAT = 512
EPS = 1e-6
ROPE_BASE = 10000.0

H_LOC = 4          # heads per core
N_CORES = 8
DLOC = H_LOC * HD  # 512, per-core proj contraction size

ST_N = S // 128    # 16 s-tiles
KT_N = D // 128    # 16 k-tiles for GEMM-A
QB = 512           # attention q-block width
NB = 512           # proj output block width

A_QW = H_LOC * HD             # 512  q columns in A
A_RW = H_LOC * ROPE           # 256  k_rope columns in A
A_W = A_QW + A_RW + LAT       # 1280 total A columns
KV_W = H_LOC * NOPE + H_LOC * HD   # 768 kv columns

X8_CHUNK = 256                # s-columns of x^T per streamed chunk
X8_N = S // X8_CHUNK          # 8 chunks
ST_PER_CHUNK = X8_CHUNK // 128  # 2

MULT = mybir.AluOpType.mult
ADD = mybir.AluOpType.add
SUB = mybir.AluOpType.subtract
EXPF = mybir.ActivationFunctionType.Exp
LNF = mybir.ActivationFunctionType.Ln
SQRTF = mybir.ActivationFunctionType.Sqrt
SQF = mybir.ActivationFunctionType.Square
AXX = mybir.AxisListType.X
AXXY = mybir.AxisListType.XY

_PROGRAM_CACHE = {}


def _build_program():
    nc = bacc.Bacc(None, target_bir_lowering=False, debug=True)

    # ---- DRAM I/O ----
    xT8 = nc.dram_tensor("xT8", [X8_N, D, X8_CHUNK], BF16, kind="ExternalInput")
    w_a = nc.dram_tensor("w_a", [D, A_W], BF16, kind="ExternalInput")
    w_up = nc.dram_tensor("w_up", [LAT, KV_W], BF16, kind="ExternalInput")
    w_p = nc.dram_tensor("w_p", [DLOC, D], BF16, kind="ExternalInput")
    cos4 = nc.dram_tensor("cos4", [S, H_LOC, ROPE // 2], BF16, kind="ExternalInput")
    sin4 = nc.dram_tensor("sin4", [S, H_LOC, ROPE // 2], BF16, kind="ExternalInput")
    masks = nc.dram_tensor("masks", [128, 4, QB], BF16, kind="ExternalInput")
    gain13 = nc.dram_tensor("gain13", [128, 13], F32, kind="ExternalInput")
    ones_in = nc.dram_tensor("ones_in", [128, 1], BF16, kind="ExternalInput")
    ident_in = nc.dram_tensor("ident_in", [128, 128], BF16, kind="ExternalInput")
    out = nc.dram_tensor("out", [S, D], F32, kind="ExternalOutput")

    with tile.TileContext(nc) as tc, ExitStack() as top:
        const = top.enter_context(tc.tile_pool(name="const", bufs=1))
        big = top.enter_context(tc.tile_pool(name="big", bufs=1))

        # --- resident weights/constants, load order = consumption order ---
        wa_sb = const.tile([128, KT_N, A_W], BF16)
        wa_r = w_a[:].rearrange("(k p) n -> p k n", p=128)
        nc.sync.dma_start(out=wa_sb[:, 0, :], in_=wa_r[:, 0, :])
        wup_sb = const.tile([128, LAT // 128, KV_W], BF16)
        nc.sync.dma_start(out=wup_sb[:], in_=w_up[:].rearrange("(k p) n -> p k n", p=128))
        cos_sb = const.tile([128, ST_N, H_LOC, ROPE // 2], BF16)
        nc.sync.dma_start(out=cos_sb[:], in_=cos4[:].rearrange("(t p) h f -> p t h f", p=128))
        sin_sb = const.tile([128, ST_N, H_LOC, ROPE // 2], BF16)
        nc.sync.dma_start(out=sin_sb[:], in_=sin4[:].rearrange("(t p) h f -> p t h f", p=128))
        gain_sb = const.tile([128, 13], F32)
        nc.sync.dma_start(out=gain_sb[:], in_=gain13[:])
        ones_sb = const.tile([128, 1], BF16)
        nc.sync.dma_start(out=ones_sb[:], in_=ones_in[:])
        ident_sb = const.tile([128, 128], BF16)
        nc.sync.dma_start(out=ident_sb[:], in_=ident_in[:])
        eps_sb = const.tile([128, 1], F32)
        nc.vector.memset(eps_sb[:], EPS)
        # loaded late (only needed by attention/proj phase)
        mask_sb = const.tile([128, 4, QB], BF16)
        wp_sb = const.tile([128, H_LOC, D], BF16)

        # --- persistent activations (head-dim-major) ---
        QT = big.tile([128, H_LOC, S], BF16)   # [d, h, q]
        KT = big.tile([128, H_LOC, S], BF16)   # [d, h, k] (0:64 nope, 64:128 rope)
        V = big.tile([128, ST_N, H_LOC * HD], BF16)  # [s%128, s//128, d_loc]

        # ===== phase 1 scratch =====
        p12 = ExitStack()
        ckvT_pool = p12.enter_context(tc.tile_pool(name="ckvT_pool", bufs=1))
        x8p = p12.enter_context(tc.tile_pool(name="x8p", bufs=2))
        xq0 = x8p.tile([128, KT_N, X8_CHUNK], BF16, tag="x8")
        nc.sync.dma_start(out=xq0[:], in_=xT8[0].rearrange("(k p) s -> p k s", p=128))
        for kt in range(1, KT_N):
            nc.sync.dma_start(out=wa_sb[:, kt, :], in_=wa_r[:, kt, :])
        scr = p12.enter_context(tc.tile_pool(name="scr", bufs=2))
        jnk = p12.enter_context(tc.tile_pool(name="jnk", bufs=2))
        ckvT = ckvT_pool.tile([128, LAT // 128, S], BF16)  # [lat, lt, s]

        def rope_rot(xn, cos_ap, sin_ap, out1, out2):
            """out1 = x1*c + x2*s ; out2 = x2*c - x1*s  (4-head wide)."""
            RH = ROPE // 2
            x1 = xn[:, :, 0:RH]
            x2 = xn[:, :, RH:ROPE]
            t1 = scr.tile([128, H_LOC, RH], F32, tag="t1")
            t2 = scr.tile([128, H_LOC, RH], F32, tag="t2")
            nc.vector.tensor_tensor(t1[:], x1, cos_ap, MULT)
            nc.vector.tensor_tensor(t2[:], x2, sin_ap, MULT)
            nc.vector.tensor_tensor(out1, t1[:], t2[:], ADD)
            t3 = scr.tile([128, H_LOC, RH], F32, tag="t3")
            t4 = scr.tile([128, H_LOC, RH], F32, tag="t4")
            nc.vector.tensor_tensor(t3[:], x2, cos_ap, MULT)
            nc.vector.tensor_tensor(t4[:], x1, sin_ap, MULT)
            nc.vector.tensor_tensor(out2, t3[:], t4[:], SUB)

        def rsqrt_act(dst, src, n):
            """dst = 1/sqrt(src/n + eps): ACT Sqrt then fast DVE reciprocal."""
            nc.scalar.activation(dst, src, SQRTF, scale=1.0 / n, bias=eps_sb[:])
            nc.vector.reciprocal_approx_fast(out=dst, in_=dst)

        # ========== phase 1: GEMM-A + norms + rope + GEMM-3, per s-tile =====
        with (
            tc.tile_pool(name="psA", bufs=2, space="PSUM") as psA,
            tc.tile_pool(name="psT", bufs=2, space="PSUM") as psT,
        ):
            for e in range(X8_N):
                if e == 0:
                    xq = xq0
                else:
                    xq = x8p.tile([128, KT_N, X8_CHUNK], BF16, tag="x8")
                    nc.sync.dma_start(
                        out=xq[:], in_=xT8[e].rearrange("(k p) s -> p k s", p=128))
                for st2 in range(ST_PER_CHUNK):
                    ST = e * ST_PER_CHUNK + st2
                    s0 = ST * 128
                    aps = psA.tile([128, A_W], F32, tag="A")
                    for kt in range(KT_N):
                        lhs = xq[:, kt, st2 * 128:(st2 + 1) * 128]
                        for c0, c1 in ((0, 512), (512, 1024), (1024, 1280)):
                            nc.tensor.matmul(
                                aps[:, c0:c1], lhs, wa_sb[:, kt, c0:c1],
                                start=(kt == 0), stop=(kt == KT_N - 1))

                    # ---- evict + batched stats ----
                    asb = scr.tile([128, A_W], F32, tag="asb")
                    nc.scalar.copy(asb[:], aps[:])
                    junk = jnk.tile([128, A_W], BF16, tag="junk")
                    nc.scalar.activation(junk[:], aps[:], SQF)
                    rs13 = scr.tile([128, 13], F32, tag="rs13")
                    nc.vector.tensor_reduce(
                        rs13[:, 0:12],
                        junk[:, 0:768].rearrange("p (g c) -> p g c", c=64),
                        AXX, ADD)
                    nc.vector.tensor_reduce(
                        rs13[:, 12:13],
                        junk[:, 768:1280].rearrange("p (g c) -> p g c", c=64),
                        AXXY, ADD)
                    rsqrt_act(rs13[:, 0:12], rs13[:, 0:12], 64)
                    rsqrt_act(rs13[:, 12:13], rs13[:, 12:13], LAT)
                    nc.vector.tensor_tensor(rs13[:], rs13[:], gain_sb[:], MULT)

                    # ---- apply norms ----
                    nrm = scr.tile([128, 768], BF16, tag="nrm")
                    nc.vector.tensor_tensor(
                        nrm[:].rearrange("p (g c) -> p g c", c=64),
                        asb[:, 0:768].rearrange("p (g c) -> p g c", c=64),
                        rs13[:, 0:12].to_broadcast([128, 12, 64]), MULT)
                    cv = scr.tile([128, LAT], BF16, tag="cv")
                    nc.vector.tensor_scalar(
                        cv[:], asb[:, 768:1280], rs13[:, 12:13], None, MULT)

                    # ---- rope ----
                    nrmq = nrm[:, 0:512].rearrange("p (h t c) -> p h t c", t=2, c=64)
                    qno = nrmq[:, :, 0, :]
                    qro = nrmq[:, :, 1, :]
                    kro = nrm[:, 512:768].rearrange("p (h c) -> p h c", c=64)
                    qrot = scr.tile([128, H_LOC, ROPE], BF16, tag="qrot")
                    krot = scr.tile([128, H_LOC, ROPE], BF16, tag="krot")
                    RH = ROPE // 2
                    rope_rot(qro, cos_sb[:, ST], sin_sb[:, ST],
                             qrot[:, :, 0:RH], qrot[:, :, RH:ROPE])
                    rope_rot(kro, cos_sb[:, ST], sin_sb[:, ST],
                             krot[:, :, 0:RH], krot[:, :, RH:ROPE])

                    # ---- transposes (q/krope/ckv) ----
                    for h in range(H_LOC):
                        tq = psT.tile([128, 128], BF16, tag="tq")
                        nc.tensor.transpose(tq[0:64, :], qno[:, h, :], ident_sb[:])
                        nc.tensor.transpose(tq[64:128, :], qrot[:, h, :], ident_sb[:])
                        nc.scalar.copy(QT[:, h, s0:s0 + 128], tq[:])
                        tk = psT.tile([128, 128], BF16, tag="tq")
                        nc.tensor.transpose(tk[64:128, :], krot[:, h, :], ident_sb[:])
                        nc.scalar.copy(KT[64:128, h, s0:s0 + 128], tk[64:128, :])
                    for lt in range(LAT // 128):
                        tcv = psT.tile([128, 128], BF16, tag="tq")
                        nc.tensor.transpose(
                            tcv[:], cv[:, lt * 128:(lt + 1) * 128], ident_sb[:])
                        nc.scalar.copy(ckvT[:, lt, s0:s0 + 128], tcv[:])

        # ================= phase 2: GEMM-3 (kv_up) =================
        with (
            tc.tile_pool(name="psKV", bufs=2, space="PSUM") as psKV,
            tc.tile_pool(name="psT2", bufs=2, space="PSUM") as psT2,
        ):
            for ST in range(ST_N):
                s0 = ST * 128
                kvps = psKV.tile([128, KV_W], F32, tag="KV")
                for lt in range(LAT // 128):
                    lhs = ckvT[:, lt, s0:s0 + 128]
                    for c0, c1 in ((0, 512), (512, 768)):
                        nc.tensor.matmul(
                            kvps[:, c0:c1], lhs, wup_sb[:, lt, c0:c1],
                            start=(lt == 0), stop=(lt == LAT // 128 - 1))
                # k_nope batched norm + transpose into KT[0:64]
                kvev = scr.tile([128, 256], F32, tag="kvev")
                nc.scalar.copy(kvev[:], kvps[:, 0:256])
                junkk = jnk.tile([128, 256], BF16, tag="junkk")
                nc.scalar.activation(junkk[:], kvps[:, 0:256], SQF)
                rsk = scr.tile([128, 4], F32, tag="rsk")
                nc.vector.tensor_reduce(
                    rsk[:], junkk[:].rearrange("p (g c) -> p g c", c=64),
                    AXX, ADD)
                rsqrt_act(rsk[:], rsk[:], 64)
                knrm = scr.tile([128, H_LOC, NOPE], BF16, tag="knrm")
                nc.vector.tensor_tensor(
                    knrm[:],
                    kvev[:].rearrange("p (g c) -> p g c", c=64),
                    rsk[:].to_broadcast([128, 4, 64]), MULT)
                for h in range(H_LOC):
                    tkn = psT2.tile([128, 128], BF16, tag="tkn")
                    nc.tensor.transpose(tkn[0:64, :], knrm[:, h, :], ident_sb[:])
                    nc.vector.tensor_copy(
                        KT[0:64, h, s0:s0 + 128], tkn[0:64, :])
                # V evict (ACT copy, table-free)
                nc.scalar.copy(V[:, ST, :], kvps[:, H_LOC * NOPE:KV_W])

        p12.close()

        # late const loads (attention/proj only)
        nc.sync.dma_start(out=mask_sb[:], in_=masks[:])
        nc.sync.dma_start(out=wp_sb[:], in_=w_p[:].rearrange("(k p) n -> p k n", p=128))

        # ============ phase 3: attention + out projection (interleaved) ======
        yT_pool = top.enter_context(tc.tile_pool(name="yT_pool", bufs=1))
        yT = yT_pool.tile([128, H_LOC, S], BF16)  # [d, h, q]
        inv_sqrt_hd = 1.0 / math.sqrt(HD)
        with (
            tc.tile_pool(name="pP", bufs=6) as pP,
            tc.tile_pool(name="pR", bufs=2) as pR,
            tc.tile_pool(name="psS", bufs=4, space="PSUM") as psS,
            tc.tile_pool(name="psL", bufs=2, space="PSUM") as psL,
            tc.tile_pool(name="psY", bufs=2, space="PSUM") as psY,
        ):
            for j in range(S // QB):
                q0 = j * QB
                nkt = (q0 + QB) // 128
                for h in range(H_LOC):
                    lps = psL.tile([1, QB], F32, tag="L")
                    yps = psY.tile([128, QB], F32, tag="Y")
                    for kt in range(nkt):
                        sps = psS.tile([128, QB], F32, tag="Ssc")
                        nc.tensor.matmul(
                            sps[:], KT[:, h, kt * 128:(kt + 1) * 128],
                            QT[:, h, q0:q0 + QB], start=True, stop=True)
                        P = pP.tile([128, QB], BF16, tag="P")
                        nc.scalar.activation(P[:], sps[:], EXPF, scale=inv_sqrt_hd)
                        d_idx = kt - (q0 // 128)
                        if d_idx >= 0:
                            nc.vector.tensor_tensor(
                                P[:], P[:], mask_sb[:, d_idx, :], MULT)
                        nc.tensor.matmul(
                            lps[:], ones_sb[:], P[:],
                            start=(kt == 0), stop=(kt == nkt - 1))
                        nc.tensor.matmul(
                            yps[:], V[:, kt, h * HD:(h + 1) * HD], P[:],
                            start=(kt == 0), stop=(kt == nkt - 1))
                    r = pR.tile([1, QB], F32, tag="r")
                    nc.vector.reciprocal_approx_fast(out=r[:], in_=lps[:])
                    rbc = pR.tile([128, QB], F32, tag="rbc")
                    nc.gpsimd.partition_broadcast(rbc[:], r[:])
                    nc.vector.tensor_tensor(
                        yT[:, h, q0:q0 + QB], yps[:], rbc[:], MULT)

        # ============ phase 4: out projection (LDW-amortized order) ==========
        with (
            tc.tile_pool(name="pO", bufs=4) as pO,
            tc.tile_pool(name="psO", bufs=8, space="PSUM") as psO,
        ):
            for ST in range(ST_N):
                s0 = ST * 128
                otiles = []
                for _nb in range(D // NB):
                    ot = psO.tile([128, NB], F32, tag="O")
                    otiles.append(ot)
                for h in range(H_LOC):
                    for nb in range(D // NB):
                        nc.tensor.matmul(
                            otiles[nb][:], yT[:, h, s0:s0 + 128],
                            wp_sb[:, h, nb * NB:(nb + 1) * NB],
                            start=(h == 0), stop=(h == H_LOC - 1))
                for nb in range(D // NB):
                    osb = pO.tile([128, NB], F32, tag="osb")
                    nc.vector.tensor_copy(osb[:], otiles[nb][:])
                    nc.sync.dma_start(
                        out=out[s0:s0 + 128, nb * NB:(nb + 1) * NB], in_=osb[:])
    nc.compile()
    return nc


def _prep_inputs(x, w_q_krope, w_kv_down, w_kv_up, w_proj, q_gain):
    """Build the 8 per-core input maps (host-side sharding)."""
    inv_freq = ROPE_BASE ** (-np.arange(0, ROPE, 2, dtype=np.float32) / ROPE)
    t = np.arange(S, dtype=np.float32)
    freqs = np.outer(t, inv_freq)                      # (S, 32)
    cos4 = np.ascontiguousarray(np.broadcast_to(
        np.cos(freqs)[:, None, :], (S, H_LOC, ROPE // 2))).astype(BF)
    sin4 = np.ascontiguousarray(np.broadcast_to(
        np.sin(freqs)[:, None, :], (S, H_LOC, ROPE // 2))).astype(BF)

    kk = np.arange(128)[:, None, None]
    dd = np.arange(4)[None, :, None]
    qq = np.arange(QB)[None, None, :]
    masks = (kk + 128 * dd <= qq).astype(BF)           # [128, 4, QB]

    ones_in = np.ones((128, 1), dtype=BF)
    ident_in = np.eye(128, dtype=np.float32).astype(BF)

    # x^T per batch, chunked: [X8_N, D, X8_CHUNK]
    xT_chunks = []
    for b in range(B):
        xT = np.ascontiguousarray(x[b].T).astype(BF)   # [D, S]
        xT_chunks.append(np.ascontiguousarray(
            xT.reshape(D, X8_N, X8_CHUNK).transpose(1, 0, 2)))

    in_maps = []
    for c in range(N_CORES):
        b = c // H_LOC
        hg = c % H_LOC
        heads = [hg * H_LOC + i for i in range(H_LOC)]
        w_a = np.concatenate(
            [w_q_krope[:, h * HD:(h + 1) * HD] for h in heads]
            + [w_q_krope[:, D + h * ROPE:D + (h + 1) * ROPE] for h in heads]
            + [w_kv_down], axis=1).astype(BF)           # [D, 1280]
        w_up = np.concatenate(
            [w_kv_up[:, h * NOPE:(h + 1) * NOPE] for h in heads]
            + [w_kv_up[:, NOPE * H + h * HD:NOPE * H + (h + 1) * HD]
               for h in heads], axis=1).astype(BF)      # [LAT, 768]
        w_p = w_proj[hg * DLOC:(hg + 1) * DLOC, :].astype(BF)   # [512, D]
        g = q_gain[heads].astype(np.float32)
        g13 = np.concatenate([np.repeat(g, 2), np.ones(5, np.float32)])
        gain13 = np.ascontiguousarray(
            np.broadcast_to(g13[None, :], (128, 13))).astype(np.float32)
        in_maps.append({
            "xT8": xT_chunks[b],
            "w_a": np.ascontiguousarray(w_a),
            "w_up": np.ascontiguousarray(w_up),
            "w_p": np.ascontiguousarray(w_p),
            "cos4": cos4, "sin4": sin4, "masks": masks,
            "gain13": gain13,
            "ones_in": ones_in, "ident_in": ident_in,
        })
    return in_maps


def kernel(x, w_q_krope, w_kv_down, w_kv_up, w_proj, q_gain, **_unused):
    x = np.asarray(x, dtype=np.float32)
    w_q_krope = np.asarray(w_q_krope, dtype=np.float32)
    w_kv_down = np.asarray(w_kv_down, dtype=np.float32)
    w_kv_up = np.asarray(w_kv_up, dtype=np.float32)
    w_proj = np.asarray(w_proj, dtype=np.float32)
    q_gain = np.asarray(q_gain, dtype=np.float32)

    if "nc" not in _PROGRAM_CACHE:
        _PROGRAM_CACHE["nc"] = _build_program()
    nc = _PROGRAM_CACHE["nc"]

    in_maps = _prep_inputs(x, w_q_krope, w_kv_down, w_kv_up, w_proj, q_gain)
    res = run_bass_kernel_spmd(nc, in_maps, list(range(N_CORES)))

    out = np.zeros((B, S, D), dtype=np.float32)
    for c in range(N_CORES):
        out[c // H_LOC] += res.results[c]["out"]
    return out
